# revision 43
# baseline (speedup 1.0000x reference)
"""Trainium2 Bass kernel for nn_Attention_41686952575399 (sparse attention).

Sharding: data-parallel over batch (2 groups of 4 cores) x tensor-parallel
over heads (4 heads per core).  Each core computes a full-width PARTIAL
output projection over its 4 local heads; the host sums the 4 partials per
batch element (no device collective at all).

Heads are processed as two STACKED PAIRS: head 2p on SBUF partitions 0..63
and head 2p+1 on 64..127, so nearly every op runs with all 128 lanes.  The
compression conv uses block-diagonal weights (host-prepared), the conv's
positional/bias term is folded into a host-precomputed rank-1 matmul, the
compressed V-conv is emitted directly in [block, dim] orientation (no PE
transposes), and the window branch stacks its P@V output with a
tile_position column offset.  All matmul operands are bf16 (psum stays
fp32).  Softmax normalization: per-branch denominators land on psum
partitions 0..3 via indicator-column matmuls BEFORE P@V, one approximate
reciprocal + gate fold per pair, a PE matmul broadcasts the gates along
partitions, P is scaled in place, and both branches accumulate into a
single psum tile — there is no separate mix stage and no [1,N] row math.
"""
import os
import sys

sys.path.insert(0, "/opt/trn_rl_repo")

DEBUG = os.environ.get("BASSK_DEBUG") == "1"

import ml_dtypes
import numpy as np

from concourse import bacc, bass, mybir, tile
from concourse.bass_utils import run_bass_kernel_spmd

B, N, DIM = 2, 1024, 1024
H, DH = 16, 64
WIN, CB = 64, 16
NB = N // CB               # 64 compressed blocks
HPC = 4                    # heads per core
NCORES = 8
F32 = mybir.dt.float32
BF = mybir.dt.bfloat16
BF_NP = ml_dtypes.bfloat16
NEG = -1e30
EPS = float(np.finfo(np.float32).eps)
SCALE = float(DH ** -0.5)
NF = 3 * HPC * DH + 3      # 771 projection output features (q,k,v slices + Ws)
KT = NB + 2                # 66: conv block columns + 2 zero pad columns

AL = mybir.AluOpType
AF = mybir.ActivationFunctionType

# Window-branch segments per 512-col psum channel: (col0, key_tile, pw_off,
# width).  Key tile kt holds scores for queries 128*kt .. 128*kt+255; each
# query column is covered by (up to) two key tiles which accumulate in psum.
WSEGS = {
    0: [(0, 0, 0, 256), (128, 1, 0, 256), (256, 2, 0, 256), (384, 3, 0, 128)],
    1: [(0, 3, 128, 128), (0, 4, 0, 256), (128, 5, 0, 256), (256, 6, 0, 256),
        (384, 7, 0, 128)],
}


def build_program() -> bass.Bass:
    nc = bacc.Bacc("TRN2", target_bir_lowering=False, debug=False,
                   num_devices=NCORES)

    inpT_d = nc.dram_tensor("inpT", [DIM, N], BF, kind="ExternalInput")
    wall_d = nc.dram_tensor("w_all", [DIM, NF], BF, kind="ExternalInput")
    # block-diagonal grouped-conv weights: [64u+i, pair, t, 64u+o]
    cwk_d = nc.dram_tensor("cw_k2", [128, 2, CB, 128], BF, kind="ExternalInput")
    cwv_d = nc.dram_tensor("cw_v2", [128, 2, CB, 128], BF, kind="ExternalInput")
    # rank-1 conv constants: sum_t cw . pos + bias, per (pair, stacked dim)
    ckpos_d = nc.dram_tensor("ckpos", [1, 2, 128], BF, kind="ExternalInput")
    cvpos_d = nc.dram_tensor("cvpos", [1, 2, 128], BF, kind="ExternalInput")
    bs_d = nc.dram_tensor("bs_t", [3, 1], F32, kind="ExternalInput")
    wout_d = nc.dram_tensor("woutP", [128, 2, 8, 128], BF, kind="ExternalInput")
    ones_d = nc.dram_tensor("ones_c", [128, 8], BF, kind="ExternalInput")
    onesr_d = nc.dram_tensor("onesr", [1, 128], BF, kind="ExternalInput")
    ident_d = nc.dram_tensor("ident_c", [128, 128], BF, kind="ExternalInput")
    # packed selector constants (see _prep_core)
    selcst_d = nc.dram_tensor("selcst", [128, 12], BF, kind="ExternalInput")
    bccst_d = nc.dram_tensor("bccst", [4, 384], BF, kind="ExternalInput")
    sel34_d = nc.dram_tensor("sel34", [3, 4], BF, kind="ExternalInput")
    outT_d = nc.dram_tensor("outT", [DIM, N], BF, kind="ExternalOutput")
    dbg = {}
    if DEBUG:
        dbg["s"] = nc.dram_tensor("dbg_s", [1, N], F32, kind="ExternalOutput")
        dbg["w3"] = nc.dram_tensor("dbg_w3", [3, N], F32, kind="ExternalOutput")
        dbg["qkv2"] = nc.dram_tensor("dbg_qkv2", [128, 6, N + 2 * CB], F32,
                                     kind="ExternalOutput")
        dbg["pc2"] = nc.dram_tensor("dbg_pc2", [128, N], F32,
                                    kind="ExternalOutput")
        dbg["pw0"] = nc.dram_tensor("dbg_pw0", [128, 8, 256], F32,
                                    kind="ExternalOutput")
        dbg["gt"] = nc.dram_tensor("dbg_gt", [4, N], F32, kind="ExternalOutput")
        dbg["ckbd"] = nc.dram_tensor("dbg_ckbd", [128, 128], F32,
                                     kind="ExternalOutput")
        dbg["cvbd"] = nc.dram_tensor("dbg_cvbd", [128, 128], F32,
                                     kind="ExternalOutput")
        dbg["cmb2"] = nc.dram_tensor("dbg_cmb2", [128, 2, N], F32,
                                     kind="ExternalOutput")

    with tile.TileContext(nc) as tc:
        _body(nc, tc, inpT_d, wall_d, cwk_d, cwv_d, ckpos_d, cvpos_d,
              bs_d, wout_d, outT_d, ones_d, onesr_d, ident_d,
              selcst_d, bccst_d, sel34_d, dbg)
    nc.compile()
    return nc


def _body(nc, tc, inpT_d, wall_d, cwk_d, cwv_d, ckpos_d, cvpos_d,
          bs_d, wout_d, outT_d, ones_d, onesr_d, ident_d,
          selcst_d, bccst_d, sel34_d, dbg):
    mm = nc.tensor.matmul

    # ----- long-lived constants -----------------------------------------
    const_cm = tc.tile_pool(name="const", bufs=1)
    const = const_cm.__enter__()
    ones_col = const.tile([128, 1], BF, name="ones_col")
    onesr = const.tile([1, 128], BF, name="onesr")
    ident = const.tile([128, 128], BF, name="ident")
    cmask2 = const.tile([128, N], F32, name="cmask2")
    wmask = const.tile([128, 256], F32, name="wmask")
    bs_sb = const.tile([3, 1], F32, name="bs_sb")
    ckpos = const.tile([1, 2, 128], BF, name="ckpos")
    cvpos = const.tile([1, 2, 128], BF, name="cvpos")
    s_row = const.tile([1, N], F32, name="s_row")
    s_tmp = const.tile([1, N], F32, name="s_tmp")
    eps_sb = const.tile([1, 1], F32, name="eps_sb")
    s_bcast = const.tile([128, N], F32, name="s_bcast")
    w3r = const.tile([3, N], F32, name="w3r")
    w3_sb = const.tile([3, N], F32, name="w3_sb")
    w3b = const.tile([3, N], BF, name="w3b")
    w34_sb = const.tile([4, N], F32, name="w34_sb")
    wout_sb = const.tile([128, 2, 8, 128], BF, name="wout_sb")
    comb2 = const.tile([128, 2, N], BF, name="comb2")
    outSB = const.tile([128, 8, N], BF, name="outSB")
    selcst = const.tile([128, 12], BF, name="selcst")
    bccst = const.tile([4, 384], BF, name="bccst")
    sel34 = const.tile([3, 4], BF, name="sel34")
    selc2 = selcst[:, 0:4]
    selw2 = [selcst[:, 4:8], selcst[:, 8:12]]
    bcc2 = bccst[:, 0:128]
    bcw2 = [bccst[:, 128:256], bccst[:, 256:384]]

    nc.gpsimd.dma_start(out=ones_col[:], in_=ones_d.ap()[:, 0:1])
    nc.gpsimd.dma_start(out=onesr[:], in_=onesr_d.ap())
    nc.gpsimd.memset(eps_sb[:], EPS)
    nc.gpsimd.dma_start(out=ident[:], in_=ident_d.ap())
    nc.sync.dma_start(out=selcst[:], in_=selcst_d.ap())
    nc.sync.dma_start(out=bccst[:], in_=bccst_d.ap())
    nc.sync.dma_start(out=sel34[:], in_=sel34_d.ap())
    # compressed-block causal mask, stacked for a head pair: build the
    # per-head [64, N] mask on partitions 0..63, then replicate to 64..127
    # with a partition-shifting SBUF->SBUF DMA.
    nc.gpsimd.memset(cmask2[:], 0.0)
    nc.gpsimd.affine_select(out=cmask2[0:64, :], in_=cmask2[0:64, :],
                            compare_op=AL.is_ge, fill=NEG, base=-15,
                            channel_multiplier=-16, pattern=[[1, N]])
    nc.sync.dma_start(out=cmask2[64:128, :], in_=cmask2[0:64, :])
    # window mask on a [key r, query j] tile: visible iff r <= j <= r+63
    nc.gpsimd.memset(wmask[:], 0.0)
    nc.gpsimd.affine_select(out=wmask[:], in_=wmask[:], compare_op=AL.is_ge,
                            fill=NEG, base=0, channel_multiplier=-1,
                            pattern=[[1, 256]])
    nc.gpsimd.affine_select(out=wmask[:], in_=wmask[:], compare_op=AL.is_ge,
                            fill=NEG, base=63, channel_multiplier=1,
                            pattern=[[-1, 256]])

    nc.sync.dma_start(out=bs_sb[:], in_=bs_d.ap())
    nc.sync.dma_start(out=ckpos[:], in_=ckpos_d.ap())
    nc.sync.dma_start(out=cvpos[:], in_=cvpos_d.ap())
    nc.gpsimd.dma_start(out=wout_sb[:], in_=wout_d.ap())

    # ----- stage 1+2: RMS stats + fused qkv/Ws projection ---------------
    # qkv2 slots: 0,1 = q pair0/1; 2,3 = k; 4,5 = v.  Partitions 0..63 =
    # even head of the pair, 64..127 = odd head.  Columns N.. stay zero so
    # the conv's 66-wide slabs read zeros there.
    qkv2, qkv2_free = tc.tile([128, 6, N + 2 * CB], BF, name="qkv2")
    x_sb, x_free = tc.tile([128, 8, N], BF, name="x_sb")
    w_sb, w_free = tc.tile([128, 8, NF], BF, name="w_sb")

    qs = [nc.gpsimd, nc.sync, nc.scalar]
    for k in range(8):
        qs[k % 3].dma_start(out=x_sb[:, k, :],
                            in_=inpT_d.ap()[128 * k:128 * (k + 1), :])
        qs[(k + 1) % 3].dma_start(out=w_sb[:, k, :],
                                  in_=wall_d.ap()[128 * k:128 * (k + 1), :])
    nc.gpsimd.memset(qkv2[:, :, N:N + 2 * CB], 0.0)

    psP_cm = tc.tile_pool(name="psP", bufs=4, space="PSUM")
    psP = psP_cm.__enter__()
    sqp_cm = tc.tile_pool(name="sqp", bufs=2)
    sqp = sqp_cm.__enter__()

    # sum of squares over dim via ones-matmul on squared tiles
    ps_s = [psP.tile([1, 512], F32, name=f"ps_s{ch}", bufs=1) for ch in range(2)]
    for k in range(8):
        sq = sqp.tile([128, N], BF, name="sq")
        if k % 2 == 0:
            nc.scalar.activation(sq[:], x_sb[:, k, :], AF.Square)
        else:
            nc.vector.tensor_tensor(sq[:], x_sb[:, k, :], x_sb[:, k, :], op=AL.mult)
        for ch in range(2):
            mm(ps_s[ch][:], ones_col[:], sq[:, 512 * ch:512 * (ch + 1)],
               start=(k == 0), stop=(k == 7))
    for ch in range(2):
        nc.scalar.activation(s_tmp[0:1, 512 * ch:512 * (ch + 1)], ps_s[ch][:],
                             AF.Sqrt, bias=eps_sb[:], scale=1.0 / DIM)
    nc.vector.reciprocal_approx_fast(out=s_row[:], in_=s_tmp[:])
    nc.gpsimd.partition_broadcast(s_bcast[:], s_row[:])

    # qkv2[:, f, t] = (W_eff.T @ inpT)[128f + p, t] * s[t] — the natural
    # 128-column tile of the projection IS the stacked head-pair layout.
    for f in range(7):
        for ch in range(2):
            pp = psP.tile([128, 512], F32, name="pp")
            sl = slice(512 * ch, 512 * (ch + 1))
            M = 128 if f < 6 else 3
            for k in range(8):
                mm(pp[:M, :], w_sb[:, k, 128 * f:128 * f + M],
                   x_sb[:, k, sl], start=(k == 0), stop=(k == 7))
            if f < 6:
                nc.vector.tensor_tensor(qkv2[:, f, sl], pp[:, :],
                                        s_bcast[:, sl], op=AL.mult)
            else:
                nc.vector.tensor_tensor(w3r[:, sl], pp[:3, :],
                                        s_bcast[:3, sl], op=AL.mult)
    nc.scalar.activation(w3_sb[:], w3r[:], AF.Sigmoid, bias=bs_sb[:])
    nc.scalar.copy(w3b[:], w3_sb[:])
    # gate pattern rows (wc, wc, ww, ww) via a tiny PE broadcast
    for ch in range(2):
        sl = slice(512 * ch, 512 * (ch + 1))
        p34 = psP.tile([4, 512], F32, name="p34", bufs=1)
        mm(p34[:], sel34[:], w3b[:, sl], start=True, stop=True)
        nc.vector.tensor_copy(w34_sb[:, sl], p34[:])
    if DEBUG:
        nc.sync.dma_start(out=dbg["s"].ap(), in_=s_row[:])
        nc.sync.dma_start(out=dbg["w3"].ap(), in_=w3_sb[:])

    sqp_cm.__exit__(None, None, None)
    psP_cm.__exit__(None, None, None)
    w_free()
    x_free()

    # ----- stage 3-6: attention, one stacked head-pair at a time ---------
    cwp_cm = tc.tile_pool(name="cwp", bufs=1)
    cwp = cwp_cm.__enter__()
    cwk_sb = cwp.tile([128, 2, CB, 128], BF, name="cwk_sb")
    cwv_sb = cwp.tile([128, 2, CB, 128], BF, name="cwv_sb")
    nc.gpsimd.dma_start(out=cwk_sb[:], in_=cwk_d.ap())
    nc.gpsimd.dma_start(out=cwv_sb[:], in_=cwv_d.ap())

    psA_cm = tc.tile_pool(name="psA", bufs=4, space="PSUM")
    psA = psA_cm.__enter__()
    psB_cm = tc.tile_pool(name="psB", bufs=2, space="PSUM")
    psB = psB_cm.__enter__()
    psO_cm = tc.tile_pool(name="psO", bufs=2, space="PSUM")
    psO = psO_cm.__enter__()
    pat_cm = tc.tile_pool(name="attn", bufs=2)
    pat = pat_cm.__enter__()
    pat2_cm = tc.tile_pool(name="attn2", bufs=2)
    pat2 = pat2_cm.__enter__()

    # software-pipelined: both pairs' front halves (conv/scores/vnat/den/
    # gates) are emitted first, so pair 1's matmuls fill the tensor queue
    # while pair 0's normalize chain runs on DVE/ACT; then both back halves
    # (broadcast/scale/P@V/combine).
    st = {}

    def front(p):
        q2 = qkv2[:, p, 0:N]
        k2 = qkv2[:, 2 + p, 0:N]
        v2 = qkv2[:, 4 + p, 0:N]

        # compression conv (both heads at once, block-diag weights); the
        # per-t slabs are strided views straight into qkv2 (stride 16 cols)
        kbT2 = qkv2[:, 2 + p, :].rearrange("p (c t) -> p t c", t=CB)
        vbT2 = qkv2[:, 4 + p, :].rearrange("p (c t) -> p t c", t=CB)

        # K-conv in [stacked dim, block] orientation
        ps_ck2 = psA.tile([128, KT], F32, name="ps_ck2", tag="psa")
        for t in range(CB):
            mm(ps_ck2[:], cwk_sb[:, p, t, :], kbT2[:, t, :],
               start=(t == 0), stop=False)
        mm(ps_ck2[:], ckpos[:, p, :], onesr[0:1, 0:KT],
           start=False, stop=True)
        ck_bd = pat2.tile([128, 128], BF, name="ck_bd")
        nc.gpsimd.memset(ck_bd[0:64, 64:128], 0.0)
        nc.gpsimd.memset(ck_bd[64:128, 0:64], 0.0)
        nc.scalar.copy(ck_bd[0:64, 0:64], ps_ck2[0:64, 0:NB])
        nc.vector.tensor_copy(ck_bd[64:128, 64:128], ps_ck2[64:128, 0:NB])

        # V-conv directly in [block, stacked dim] orientation
        ps_cv2 = psA.tile([KT, 128], F32, name="ps_cv2", tag="psa")
        for t in range(CB):
            mm(ps_cv2[:], vbT2[:, t, :], cwv_sb[:, p, t, :],
               start=(t == 0), stop=False)
        mm(ps_cv2[:], onesr[0:1, 0:KT], cvpos[:, p, :],
           start=False, stop=True)
        cv_bd = pat2.tile([128, 128], BF, name="cv_bd")
        cv_st = pat2.tile([64, 64], BF, name="cv_st")
        nc.gpsimd.memset(cv_bd[0:64, 64:128], 0.0)
        nc.gpsimd.memset(cv_bd[64:128, 0:64], 0.0)
        nc.scalar.copy(cv_bd[0:64, 0:64], ps_cv2[0:NB, 0:64])
        nc.vector.tensor_copy(cv_st[:], ps_cv2[0:NB, 64:128])
        nc.sync.dma_start(out=cv_bd[64:128, 64:128], in_=cv_st[:])

        # probabilities
        pc2 = pat.tile([128, N], BF, name="pc2")
        for ch in range(2):
            sl = slice(512 * ch, 512 * (ch + 1))
            ps_sc = psA.tile([128, 512], F32, name="ps_sc", tag="psa")
            mm(ps_sc[:], ck_bd[:], q2[:, sl], start=True, stop=True)
            nc.vector.tensor_tensor(ps_sc[:], ps_sc[:], cmask2[:, sl], op=AL.add)
            nc.scalar.activation(pc2[:, sl], ps_sc[:], AF.Exp, scale=SCALE)

        pw = [pat.tile([128, 8, 256], BF, name=f"pw{u}") for u in range(2)]
        for u in range(2):
            hp = slice(64 * u, 64 * u + 64)
            for kt in range(8):
                nq = 256 if kt < 7 else 128
                ps_sw = psA.tile([128, 256], F32, name="ps_sw", tag="psa")
                mm(ps_sw[:, :nq], k2[hp, 128 * kt:128 * (kt + 1)],
                   q2[hp, 128 * kt:128 * kt + nq], start=True, stop=True)
                nc.vector.tensor_tensor(ps_sw[:, :nq], ps_sw[:, :nq],
                                        wmask[:, :nq], op=AL.add)
                nc.scalar.activation(pw[u][:, kt, :nq], ps_sw[:, :nq], AF.Exp,
                                     scale=SCALE)

        # v in natural [token, stacked dim] layout (via PE transpose)
        vnat2 = pat.tile([128, 8, 128], BF, name="vnat2")
        for g in range(8):
            ps_vt = psA.tile([128, 128], BF, name="ps_vt", tag="psa")
            nc.tensor.transpose(ps_vt[:], v2[:, 128 * g:128 * (g + 1)],
                                ident[:])
            if g % 2 == 0:
                nc.scalar.copy(vnat2[:, g, :], ps_vt[:])
            else:
                nc.vector.tensor_copy(vnat2[:, g, :], ps_vt[:])

        # denominators on psum partitions 0..3 via indicator matmuls
        psd = [psA.tile([4, 512], F32, name=f"psd{ch}", tag="psa")
               for ch in range(2)]
        for ch in range(2):
            sl = slice(512 * ch, 512 * (ch + 1))
            mm(psd[ch][:], selc2, pc2[:, sl], start=True, stop=False,
               skip_group_check=True)
            segs = WSEGS[ch]
            for u in range(2):
                for i, (c0, kt, off, wd) in enumerate(segs):
                    mm(psd[ch][:, c0:c0 + wd], selw2[u],
                       pw[u][:, kt, off:off + wd], start=False,
                       stop=(u == 1 and i == len(segs) - 1),
                       skip_group_check=True)
        denr = pat.tile([4, N], F32, name="denr")
        nc.scalar.copy(denr[:, 0:512], psd[0][:])
        nc.scalar.copy(denr[:, 512:N], psd[1][:])
        # tokens 0..14 see no compressed block: den==0 would blow up the
        # approx reciprocal; pc is all-zero there so any finite gate works
        nc.vector.memset(denr[0:2, 0:15], 1.0)
        g0 = pat.tile([4, N], F32, name="g0")
        nc.vector.reciprocal_approx_fast(out=g0[:], in_=denr[:])
        gt = pat.tile([4, N], BF, name="gt")
        nc.vector.tensor_tensor(gt[:], g0[:], w34_sb[:], op=AL.mult)
        st[p] = dict(pc2=pc2, pw=pw, vnat2=vnat2, cv_bd=cv_bd, gt=gt,
                     ck_bd=ck_bd)

    def back(p):
        pc2, pw, vnat2, cv_bd, gt = (st[p][k] for k in
                                     ("pc2", "pw", "vnat2", "cv_bd", "gt"))
        # broadcast gates along partitions via PE, scale P in place
        for ch in range(2):
            sl = slice(512 * ch, 512 * (ch + 1))
            gcb = psB.tile([128, 512], F32, name="gcb", tag="psb")
            mm(gcb[:], bcc2, gt[:, sl], start=True, stop=True)
            nc.vector.tensor_tensor(pc2[:, sl], pc2[:, sl], gcb[:], op=AL.mult)
        for u in range(2):
            for ch in range(2):
                sl = slice(512 * ch, 512 * (ch + 1))
                gwb = psB.tile([128, 512], F32, name="gwb", tag="psb")
                mm(gwb[:], bcw2[u], gt[:, sl], start=True, stop=True)
                for (c0, kt, off, wd) in WSEGS[ch]:
                    nc.vector.tensor_tensor(pw[u][:, kt, off:off + wd],
                                            pw[u][:, kt, off:off + wd],
                                            gwb[:, c0:c0 + wd], op=AL.mult)

        # P@V: both branches, both heads, one psum tile per channel
        oc2 = [psO.tile([128, 512], F32, name=f"oc2{ch}", tag="pso")
               for ch in range(2)]
        for ch in range(2):
            sl = slice(512 * ch, 512 * (ch + 1))
            mm(oc2[ch][:], cv_bd[:], pc2[:, sl], start=True, stop=False,
               skip_group_check=True)
            segs = WSEGS[ch]
            for u in range(2):
                tp = None if u == 0 else (0, 64)
                for i, (c0, kt, off, wd) in enumerate(segs):
                    mm(oc2[ch][64 * u:64 * u + 64, c0:c0 + wd],
                       vnat2[:, kt, 64 * u:64 * u + 64],
                       pw[u][:, kt, off:off + wd], start=False,
                       stop=(u == 1 and i == len(segs) - 1),
                       tile_position=tp, skip_group_check=True)
        nc.scalar.copy(comb2[:, p, 0:512], oc2[0][:])
        nc.vector.tensor_copy(comb2[:, p, 512:N], oc2[1][:])
        if DEBUG and p == 0:
            nc.gpsimd.dma_start(out=dbg["qkv2"].ap(), in_=qkv2[:])
            nc.gpsimd.dma_start(out=dbg["pc2"].ap(), in_=pc2[:])
            nc.gpsimd.dma_start(out=dbg["pw0"].ap(), in_=pw[0][:])
            nc.gpsimd.dma_start(out=dbg["gt"].ap(), in_=gt[:])
            nc.gpsimd.dma_start(out=dbg["ckbd"].ap(), in_=st[p]["ck_bd"][:])
            nc.gpsimd.dma_start(out=dbg["cvbd"].ap(), in_=cv_bd[:])

    front(0)
    front(1)
    back(0)
    back(1)

    pat2_cm.__exit__(None, None, None)
    pat_cm.__exit__(None, None, None)
    psO_cm.__exit__(None, None, None)
    psB_cm.__exit__(None, None, None)
    psA_cm.__exit__(None, None, None)
    cwp_cm.__exit__(None, None, None)
    qkv2_free()

    # output projection: both pair slabs accumulate in psum; copies alternate
    # ACT/DVE and each bf16 chunk streams out as soon as it is ready
    psW_cm = tc.tile_pool(name="psW", bufs=6, space="PSUM")
    psW = psW_cm.__enter__()
    for m in range(8):
        for ch in range(2):
            sl = slice(512 * ch, 512 * (ch + 1))
            po = psW.tile([128, 512], F32, name="po")
            for s in range(2):
                mm(po[:], wout_sb[:, s, m, :], comb2[:, s, sl],
                   start=(s == 0), stop=(s == 1))
            if (m + ch) % 2 == 0:
                nc.scalar.copy(outSB[:, m, sl], po[:])
            else:
                nc.vector.tensor_copy(outSB[:, m, sl], po[:])
            dq = nc.sync if (m + ch) % 2 == 0 else nc.gpsimd
            dq.dma_start(out=outT_d.ap()[128 * m:128 * (m + 1), sl],
                         in_=outSB[:, m, sl])
    psW_cm.__exit__(None, None, None)

    if DEBUG:
        nc.gpsimd.dma_start(out=dbg["cmb2"].ap(), in_=comb2[:])
    const_cm.__exit__(None, None, None)


# --------------------------------------------------------------------------
_CACHE: dict = {}


def _get_nc() -> bass.Bass:
    if "nc" not in _CACHE:
        _CACHE["nc"] = build_program()
    return _CACHE["nc"]


def _prep_core(c: int, inputs: dict) -> dict:
    b, r = c // 4, c % 4
    hs = HPC * r
    f32 = np.float32
    inp = np.asarray(inputs["inp"], f32)
    rms_w = np.asarray(inputs["rms_w"], f32)
    Wqkv = np.asarray(inputs["Wqkv"], f32)
    k_pos = np.asarray(inputs["k_pos"], f32)
    v_pos = np.asarray(inputs["v_pos"], f32)
    k_cw = np.asarray(inputs["k_cw"], f32)
    k_cb = np.asarray(inputs["k_cb"], f32)
    v_cw = np.asarray(inputs["v_cw"], f32)
    v_cb = np.asarray(inputs["v_cb"], f32)
    Ws = np.asarray(inputs["Ws"], f32)
    bs = np.asarray(inputs["bs"], f32)
    Wout = np.asarray(inputs["Wout"], f32)

    cols = [Wqkv[:, pp * H * DH + hs * DH: pp * H * DH + (hs + HPC) * DH]
            for pp in range(3)]
    # fold rms_w into the projection weights host-side (exact)
    w_all = np.ascontiguousarray(
        np.concatenate(cols + [Ws], axis=1) * rms_w[:, None])

    # block-diagonal conv weights: cw2[64u+i, p, t, 64u+o] = cw[hs+2p+u, o, i, t]
    def blockdiag(cw):
        Z = np.zeros((128, 2, CB, 128), f32)
        for p in range(2):
            for u in range(2):
                h = hs + 2 * p + u
                Z[64 * u:64 * u + 64, p, :, 64 * u:64 * u + 64] = \
                    cw[h].transpose(1, 2, 0)  # [o,i,t] -> [i,t,o]
        return Z.astype(BF_NP)

    # rank-1 conv constant: sum_{i,t} cw[h,o,i,t]*pos[h,t,i] + cb[h,o]
    def posconst(cw, pos, cb):
        Z = np.zeros((1, 2, 128), f32)
        for p in range(2):
            for u in range(2):
                h = hs + 2 * p + u
                Z[0, p, 64 * u:64 * u + 64] = (
                    np.einsum('oit,ti->o', cw[h], pos[h]) + cb[h])
        return Z.astype(BF_NP)

    # selector constants
    selcst = np.zeros((128, 12), f32)
    selcst[0:64, 0] = 1.0    # compressed den, even head -> row 0
    selcst[64:128, 1] = 1.0  # compressed den, odd head  -> row 1
    selcst[:, 4 + 2] = 1.0   # window den, head 0 -> row 2
    selcst[:, 8 + 3] = 1.0   # window den, head 1 -> row 3
    bccst = np.zeros((4, 384), f32)
    bccst[0, 0:64] = 1.0     # gate bcast: partitions 0..63  <- row 0
    bccst[1, 64:128] = 1.0   # gate bcast: partitions 64..127 <- row 1
    bccst[2, 128:256] = 1.0  # window gate head 0 <- row 2
    bccst[3, 256:384] = 1.0  # window gate head 1 <- row 3
    sel34 = np.zeros((3, 4), f32)
    sel34[0, 0:2] = 1.0      # w34 rows 0,1 = wc
    sel34[1, 2:4] = 1.0      # w34 rows 2,3 = ww

    return {
        "inpT": np.ascontiguousarray(inp[b].T).astype(BF_NP),
        "w_all": w_all.astype(BF_NP),
        "cw_k2": blockdiag(k_cw),
        "cw_v2": blockdiag(v_cw),
        "ckpos": posconst(k_cw, k_pos, k_cb),
        "cvpos": posconst(v_cw, v_pos, v_cb),
        "bs_t": np.ascontiguousarray(bs[:, None]),
        # woutP[p, s, m, j] = Wout[256r + 128s + p, 128m + j]
        "woutP": np.ascontiguousarray(
            Wout[256 * r:256 * (r + 1), :].reshape(2, 128, 8, 128)
            .transpose(1, 0, 2, 3)).astype(BF_NP),
        "ones_c": np.ones((128, 8), BF_NP),
        "onesr": np.ones((1, 128), BF_NP),
        "ident_c": np.eye(128, dtype=f32).astype(BF_NP),
        "selcst": selcst.astype(BF_NP),
        "bccst": bccst.astype(BF_NP),
        "sel34": sel34.astype(BF_NP),
    }


def kernel(**inputs) -> np.ndarray:
    nc = _get_nc()
    in_maps = [_prep_core(c, inputs) for c in range(NCORES)]
    res = run_bass_kernel_spmd(nc, in_maps, list(range(NCORES)))
    out = np.zeros((B, N, DIM), np.float32)
    for c in range(NCORES):
        b = c // 4
        out[b] += res.results[c]["outT"].astype(np.float32).T
    return out
